# revision 1
# baseline (speedup 1.0000x reference)
"""ChebConv (K=3) forward as a distributed Bass/Tile kernel on 8 trn2 NeuronCores.

Sharding (per spec hint): vertices V are sharded across the 8 cores.
  x0 = [x[0] | x[1]]            # [V, 128], feature col = b*64 + fin
  x1 = L @ x0                   # SpMM (COO, edge-parallel)
  x2' = L @ x1 - 0.5 x0         # = x2/2; the 2x is folded into W_k2
  out[b,v,:] = bias + sum_k xk[v, b*64:(b+1)*64] @ Wk'

Each core owns a row shard (V/8 rows padded to a multiple of 128 = "blocks").
SpMM per core and per 128-edge tile (all data bf16, PSUM accumulate f32):
  - gpsimd.dma_gather fetches the 256B source feature rows from a
    flat-indexed replicated table (int16 indices, 4 chunks), spread
    round-robin over 4 SWDGE queues (parallel Q7 descriptor gen).
  - The selector M[e,j] = val[e] * (lrow[e]==j) is HOST-precomputed in bf16
    and streamed via HWDGE sync-DMA (no on-chip selector build at all).
  - PE matmul M^T @ G (spmm1, row-major out) or G^T @ M (spmm2, transposed
    out) performs the scaled segmented sum into a per-block PSUM accumulator.
Both SpMMs share one M tensor (the Chebyshev 2x lives in the mix weights).
x1 shards are AllGathered (bf16) between the SpMMs. The final channel mix is
fused into the SpMM2 block loop using block-diagonal weights (both batches in
one matmul chain) plus a rank-1 bias matmul; -0.5 x0 enters SpMM2's
accumulation as a (-0.5 I) matmul.

The tile structure is computed from the actual edge data at call time (max
over cores per (block, chunk) slot) so one SPMD program fits all 8 cores.
"""

import sys

sys.path.insert(0, "/opt/trn_rl_repo")

import numpy as np
import ml_dtypes

import concourse.bass as bass
import concourse.bacc as bacc
import concourse.mybir as mybir
import concourse.tile as tile
from concourse import bass_utils
from concourse.alu_op_type import AluOpType

P = 128
F32 = mybir.dt.float32
BF16 = mybir.dt.bfloat16
I16 = mybir.dt.int16
NPBF16 = ml_dtypes.bfloat16
NQ = 4  # SWDGE queues (parallel Q7 descriptor generation)


def _cdiv(a, b):
    return -(-a // b)


# ---------------------------------------------------------------------------
# Host-side: uniform (cross-core) edge structure + per-core content arrays
# ---------------------------------------------------------------------------


class EdgeStructure:
    def __init__(self, V, ncores, sb_blocks, nchunks, rows, cols):
        assert V % ncores == 0
        self.V, self.ncores = V, ncores
        self.vsh = V // ncores
        self.nblk = _cdiv(self.vsh, P)
        self.vpad = self.nblk * P
        self.vtot = self.vpad * ncores
        self.nchunks = nchunks
        # Unequal chunk splits keep per-(block,chunk) slot averages off
        # integer tile multiples (less ceil-quantization padding).
        c = int(round(0.2806 * self.vtot / P)) * P
        c = max(P, min(32768, c))
        bounds = [0]
        for _ in range(nchunks - 1):
            bounds.append(min(bounds[-1] + c, self.vtot))
        bounds.append(self.vtot)
        self.chunk_bounds = []
        for i in range(nchunks):
            if bounds[i + 1] > bounds[i]:
                self.chunk_bounds.append((bounds[i], bounds[i + 1]))
        self.nchunks = nchunks = len(self.chunk_bounds)
        assert all(b - a <= 32768 for a, b in self.chunk_bounds)
        self.chunk_lo = np.array([a for a, _ in self.chunk_bounds], np.int64)

        rows = np.asarray(rows, np.int64)
        cols = np.asarray(cols, np.int64)
        c_of = cols // self.vsh
        flat = c_of * self.vpad + (cols - c_of * self.vsh)
        r_core = rows // self.vsh
        r_loc = rows - r_core * self.vsh
        blk = r_loc // P
        chunk = np.searchsorted(self.chunk_lo, flat, side="right") - 1

        # slot order: for sb: for chunk: for block in sb
        sb_arr = blk // sb_blocks
        bi_arr = blk % sb_blocks
        bh_arr = np.minimum(sb_blocks, self.nblk - sb_arr * sb_blocks)
        sid = sb_arr * sb_blocks * nchunks + chunk * bh_arr + bi_arr

        self.sb_blocks = sb_blocks
        self.nsb = _cdiv(self.nblk, sb_blocks)
        order = []
        for sb in range(self.nsb):
            b0 = sb * sb_blocks
            bh = min(sb_blocks, self.nblk - b0)
            for ch in range(nchunks):
                for bi in range(bh):
                    order.append((b0 + bi, ch))
        self.nslots = len(order)
        self.slot_block = np.array([b for b, _ in order], np.int64)
        self.slot_chunk = np.array([c for _, c in order], np.int64)

        counts = np.zeros((ncores, self.nslots), np.int64)
        np.add.at(counts, (r_core, sid), 1)
        T = _cdiv(np.max(counts, axis=0), P)

        # every block needs >=1 tile so its PSUM accumulator gets written
        blk_tiles = np.zeros(self.nblk, np.int64)
        np.add.at(blk_tiles, self.slot_block, T)
        for b in np.nonzero(blk_tiles == 0)[0]:
            sb, bi = b // sb_blocks, b % sb_blocks
            bh = min(sb_blocks, self.nblk - sb * sb_blocks)
            T[sb * sb_blocks * nchunks + 0 * bh + bi] = 1

        self.T = T
        self.slot_tile_base = np.concatenate(([0], np.cumsum(T)))[:-1]
        self.ntiles = int(np.sum(T))
        self.sid_of_edge = sid
        self.flat_of_edge = flat
        self.r_core_of_edge = r_core
        self.lrow_of_edge = (r_loc % P).astype(np.int64)

        # (sb, chunk) -> contiguous tile run
        self.runs = []  # per sb: list of (tile_start, ntiles, chunk)
        s = 0
        for sb in range(self.nsb):
            b0 = sb * sb_blocks
            bh = min(sb_blocks, self.nblk - b0)
            sb_runs = []
            for ch in range(nchunks):
                t0 = int(self.slot_tile_base[s])
                ntr = int(np.sum(T[s : s + bh]))
                if ntr > 0:
                    sb_runs.append((t0, ntr, ch))
                s += bh
            self.runs.append(sb_runs)
        self.max_run_tiles = max(
            nt for sb_runs in self.runs for _, nt, _ in sb_runs
        )

        tile_block = np.empty(self.ntiles, np.int64)
        for s in range(self.nslots):
            t0, ntr = self.slot_tile_base[s], T[s]
            tile_block[t0 : t0 + ntr] = self.slot_block[s]
        self.tile_block = tile_block
        self.tile_start = np.zeros(self.ntiles, bool)
        self.tile_stop = np.zeros(self.ntiles, bool)
        first, last = {}, {}
        for t in range(self.ntiles):
            b = int(tile_block[t])
            if b not in first:
                first[b] = t
            last[b] = t
        for t in first.values():
            self.tile_start[t] = True
        for t in last.values():
            self.tile_stop[t] = True

    def per_core_arrays(self, core, vals):
        """idx (int16 wrapped+replicated) and bf16 M tiles for one core."""
        sel = np.nonzero(self.r_core_of_edge == core)[0]
        sid = self.sid_of_edge[sel]
        o = np.argsort(sid, kind="stable")
        sel, sid = sel[o], sid[o]
        start = np.searchsorted(sid, np.arange(self.nslots))
        rank = np.arange(len(sid)) - start[sid]
        pos = self.slot_tile_base[sid] * P + rank
        n = self.ntiles * P
        idx = np.zeros(n, np.int16)
        idx[pos] = (
            self.flat_of_edge[sel] - self.chunk_lo[self.slot_chunk[sid]]
        ).astype(np.int16)
        idx_w = np.tile(np.ascontiguousarray(idx.reshape(-1, 16).T), (8, 1))
        # M tiles: M[t, e, lrow] = val; stored partition-major [P, nt*P]
        m = np.zeros((self.ntiles, P, P), np.float32)
        m[pos // P, pos % P, self.lrow_of_edge[sel]] = vals[sel]
        mfull = np.ascontiguousarray(
            m.astype(NPBF16).transpose(1, 0, 2).reshape(P, self.ntiles * P)
        )
        return idx_w, mfull


# ---------------------------------------------------------------------------
# Bass program (SPMD: one program, per-core data via in_maps)
# ---------------------------------------------------------------------------


def build_program(es: EdgeStructure):
    nblk, vpad, vtot, ncores = es.nblk, es.vpad, es.vtot, es.ncores
    nt, GW, SB = es.ntiles, es.max_run_tiles, es.sb_blocks

    nc = bacc.Bacc(
        "TRN2",
        target_bir_lowering=False,
        debug=False,
        num_devices=ncores,
        num_swdge_queues=NQ,
    )

    x0f = nc.dram_tensor("x0f", [vtot, P], BF16, kind="ExternalInput")
    x0t = nc.dram_tensor("x0t", [nblk, P, P], BF16, kind="ExternalInput")
    wbd = nc.dram_tensor("wbd", [3, P, P], BF16, kind="ExternalInput")
    biasbd = nc.dram_tensor("biasbd", [1, P], BF16, kind="ExternalInput")
    nhi_d = nc.dram_tensor("nhi", [P, P], BF16, kind="ExternalInput")
    ident_d = nc.dram_tensor("ident", [P, P], BF16, kind="ExternalInput")
    ones_d = nc.dram_tensor("ones1", [1, P], BF16, kind="ExternalInput")
    eidx = nc.dram_tensor("eidx", [P, nt * 8], I16, kind="ExternalInput")
    emt = nc.dram_tensor("emt", [P, nt * P], BF16, kind="ExternalInput")
    outp = nc.dram_tensor("outp", [2, vpad, 64], F32, kind="ExternalOutput")

    x1my = nc.dram_tensor("x1my", [vpad, P], BF16)
    x1full = nc.dram_tensor("x1full", [vtot, P], BF16)

    with tile.TileContext(nc) as tc:
        with (
            tc.tile_pool(name="const", bufs=1) as cpool,
            tc.tile_pool(name="gslab", bufs=8) as gpool,
            tc.tile_pool(name="mslab", bufs=6) as mpool,
            tc.tile_pool(name="ivl", bufs=6) as ipool,
            tc.tile_pool(name="xio", bufs=4) as xpool,
            tc.tile_pool(name="ostage", bufs=4) as opool,
            tc.tile_pool(name="acc", bufs=2 * SB, space="PSUM") as apool,
            tc.tile_pool(name="ptr", bufs=1, space="PSUM") as ptpool,
            tc.tile_pool(name="pmix", bufs=1, space="PSUM") as pmpool,
        ):
            nhi_s = cpool.tile([P, P], BF16, tag="nhi")
            nc.sync.dma_start(out=nhi_s[:], in_=nhi_d[:, :])
            ident_s = cpool.tile([P, P], BF16, tag="ident")
            nc.sync.dma_start(out=ident_s[:], in_=ident_d[:, :])
            ones_s = cpool.tile([1, P], BF16, tag="ones")
            nc.sync.dma_start(out=ones_s[:], in_=ones_d[:, :])
            bias_s = cpool.tile([1, P], BF16, tag="bias")
            nc.sync.dma_start(out=bias_s[:], in_=biasbd[:, :])
            wbd_s = cpool.tile([P, 3 * P], BF16, tag="wbd")
            for k in range(3):
                nc.sync.dma_start(
                    out=wbd_s[:, k * P : (k + 1) * P], in_=wbd[k, :, :]
                )

            qn = [0]

            def spmm(src_dram, layout_b, out_cb):
                for sb in range(es.nsb):
                    b0 = sb * SB
                    bh = min(SB, nblk - b0)
                    psums = {
                        b0 + bi: apool.tile(
                            [P, P], F32, tag="acc", name=f"acc{b0 + bi}"
                        )
                        for bi in range(bh)
                    }
                    for (t0, ntr, ch) in es.runs[sb]:
                        it = ipool.tile([P, GW * 8], I16, tag="idx")
                        nc.sync.dma_start(
                            out=it[:, : ntr * 8],
                            in_=eidx[:, t0 * 8 : (t0 + ntr) * 8],
                        )
                        mt = mpool.tile([P, GW * P], BF16, tag="m")
                        nc.sync.dma_start(
                            out=mt[:, : ntr * P],
                            in_=emt[:, t0 * P : (t0 + ntr) * P],
                        )
                        g = gpool.tile([P, GW * P], BF16, tag="g")
                        nidx = ntr * P
                        nc.gpsimd.dma_gather(
                            out_ap=g[:, :nidx].rearrange(
                                "p (t e) -> p t e", e=P
                            ),
                            in_ap=src_dram[
                                es.chunk_bounds[ch][0] : es.chunk_bounds[ch][1],
                                :,
                            ],
                            idxs_ap=it[:, : ntr * 8],
                            num_idxs=nidx,
                            num_idxs_reg=nidx,
                            elem_size=P,
                            single_packet=False,
                            queue_num=qn[0] % NQ,
                        )
                        qn[0] += 1
                        for tt in range(ntr):
                            t = t0 + tt
                            b = int(es.tile_block[t])
                            gt = g[:, tt * P : (tt + 1) * P]
                            mm = mt[:, tt * P : (tt + 1) * P]
                            start = bool(es.tile_start[t])
                            stop = bool(es.tile_stop[t]) and not layout_b
                            if layout_b:
                                nc.tensor.matmul(
                                    out=psums[b][:], lhsT=gt, rhs=mm,
                                    start=start, stop=stop,
                                )
                            else:
                                nc.tensor.matmul(
                                    out=psums[b][:], lhsT=mm, rhs=gt,
                                    start=start, stop=stop,
                                )
                    for bi in range(bh):
                        out_cb(b0 + bi, psums[b0 + bi])

            # ---------------- SpMM 1: x1 = L @ x0 (row-major out) --------
            def cb1(b, ps):
                xb = opool.tile([P, P], BF16, tag="x1st")
                nc.scalar.copy(out=xb[:], in_=ps[:])
                nc.sync.dma_start(
                    out=x1my[b * P : (b + 1) * P, :], in_=xb[:]
                )

            spmm(x0f, False, cb1)

            # ---------------- AllGather x1 shards ------------------------
            nc.gpsimd.collective_compute(
                "AllGather",
                AluOpType.bypass,
                replica_groups=[list(range(ncores))],
                ins=[x1my.ap().opt()],
                outs=[x1full.ap().opt()],
            )

            # -------- SpMM 2 (transposed out) + fused channel mix --------
            def cb2(b, ps):
                # ps = (L x1)^T block; add -0.5 x0^T block via (-I/2) matmul
                x0b = xpool.tile([P, P], BF16, tag="x0b")
                nc.sync.dma_start(out=x0b[:], in_=x0t[b, :, :])
                nc.tensor.matmul(
                    out=ps[:], lhsT=nhi_s[:], rhs=x0b[:],
                    start=False, stop=True,
                )
                x2b = opool.tile([P, P], BF16, tag="x2b")
                nc.scalar.copy(out=x2b[:], in_=ps[:])
                # x1^T block via PE transpose of my x1 rows
                x1b = xpool.tile([P, P], BF16, tag="x1b")
                nc.sync.dma_start(
                    out=x1b[:], in_=x1my[b * P : (b + 1) * P, :]
                )
                pt = ptpool.tile([P, P], BF16, tag="ptr")
                nc.tensor.transpose(
                    out=pt[:], in_=x1b[:], identity=ident_s[:]
                )
                x1tb = opool.tile([P, P], BF16, tag="x1tb")
                nc.scalar.copy(out=x1tb[:], in_=pt[:])
                # channel mix: out = bias + sum_k XkT^T @ Wbd_k
                pm = pmpool.tile([P, P], F32, tag="pmix")
                nc.tensor.matmul(
                    out=pm[:], lhsT=ones_s[:], rhs=bias_s[:],
                    start=True, stop=False,
                )
                for k, xk in enumerate((x0b, x1tb, x2b)):
                    nc.tensor.matmul(
                        out=pm[:],
                        lhsT=xk[:],
                        rhs=wbd_s[:, k * P : (k + 1) * P],
                        start=False,
                        stop=(k == 2),
                    )
                ob = opool.tile([P, P], F32, tag="ob")
                nc.scalar.copy(out=ob[:], in_=pm[:])
                nc.sync.dma_start(
                    out=outp[0, b * P : (b + 1) * P, :], in_=ob[:, 0:64]
                )
                nc.sync.dma_start(
                    out=outp[1, b * P : (b + 1) * P, :], in_=ob[:, 64:128]
                )

            spmm(x1full, True, cb2)

    nc.compile()
    return nc


# ---------------------------------------------------------------------------
# Host driver
# ---------------------------------------------------------------------------


def prepare(x, weight, bias, lap_vals, lap_rows, lap_cols, ncores=8,
            sb_blocks=3, nchunks=4):
    x = np.asarray(x, np.float32)
    weight = np.asarray(weight, np.float32)
    bias = np.asarray(bias, np.float32)
    lap_vals = np.asarray(lap_vals, np.float32)
    lap_rows = np.asarray(lap_rows)
    lap_cols = np.asarray(lap_cols)
    B, V, FIN = x.shape
    _, K, FOUT = weight.shape
    assert B == 2 and FIN == 64 and K == 3 and FOUT == 64

    es = EdgeStructure(V, ncores, sb_blocks, nchunks, lap_rows, lap_cols)

    x0 = np.concatenate([x[0], x[1]], axis=1)  # [V, 128] f32
    x0f = np.zeros((es.vtot, P), NPBF16)
    for c in range(ncores):
        x0f[c * es.vpad : c * es.vpad + es.vsh] = x0[
            c * es.vsh : (c + 1) * es.vsh
        ].astype(NPBF16)

    wbd = np.zeros((3, P, P), np.float32)
    for k in range(3):
        wk = weight[:, k, :] * (2.0 if k == 2 else 1.0)  # x2' = x2/2
        wbd[k, :64, :64] = wk
        wbd[k, 64:, 64:] = wk
    wbd = wbd.astype(NPBF16)
    biasbd = np.concatenate([bias, bias]).reshape(1, P).astype(NPBF16)
    nhi = (-0.5 * np.eye(P)).astype(NPBF16)
    ident = np.eye(P, dtype=np.float32).astype(NPBF16)
    ones1 = np.ones((1, P), NPBF16)

    in_maps = []
    for c in range(ncores):
        idx_w, mfull = es.per_core_arrays(c, lap_vals)
        x0t_c = np.ascontiguousarray(
            x0f[c * es.vpad : (c + 1) * es.vpad]
            .reshape(es.nblk, P, P)
            .transpose(0, 2, 1)
        )
        in_maps.append(
            {
                "x0f": x0f,
                "x0t": x0t_c,
                "wbd": wbd,
                "biasbd": biasbd,
                "nhi": nhi,
                "ident": ident,
                "ones1": ones1,
                "eidx": idx_w,
                "emt": mfull,
            }
        )

    nc = build_program(es)

    def assemble(results):
        out = np.empty((B, V, FOUT), np.float32)
        for c in range(ncores):
            o = np.asarray(results[c]["outp"]).reshape(B, es.vpad, FOUT)
            out[:, c * es.vsh : (c + 1) * es.vsh, :] = o[:, : es.vsh, :]
        return out

    return nc, in_maps, assemble, es


def kernel(x, weight, bias, lap_vals, lap_rows, lap_cols):
    nc, in_maps, assemble, es = prepare(
        x, weight, bias, lap_vals, lap_rows, lap_cols
    )
    res = bass_utils.run_bass_kernel_spmd(
        nc, in_maps, core_ids=list(range(es.ncores))
    )
    return assemble(res.results)

